# revision 36
# baseline (speedup 1.0000x reference)
"""Trainium2 Bass kernel for the BaseMemory coref scoring module.

Computes, for full inputs (M=65536 memory slots, D=768, E=20, H=64):
    score = relu(pair @ W1 + b1) @ W2 + b2, masked with ent_counter>0,
    where pair = [mem, ment, mem*ment, dist_emb, cnt_emb].

Sharding: data-parallel over the cluster dimension M across 8 NeuronCores.

Key algebraic folds (host side, O(D*H + M*D) work, no M*D*H matmul):
  - mem@W1_mem + (mem*ment)@W1_had = mem @ W  with W = W1_mem + diag(ment)@W1_had
  - the whole per-row additive term t_m = b1 + ment@W1_ment
      + dist_table[bd_m]@W1_dist + counter_table[bc_m]@W1_cnt  (only 100
    distinct values over the two 10-way buckets) is folded INTO the data
    stream:  x'_m = mem_m + Wp t_m  with  Wp = W (W^T W)^{-1}, so that
    W^T x'_m = W^T mem_m + t_m exactly.  The device then only computes
    relu(W^T x') @ W2 -- two matmuls, nothing else.
  - masking (+b2, -10000 on empty slots, trailing 0) is applied during the
    host-side gather, as is the trailing new-cluster slot.
  - x', W, relu(z) and W2 are cast to bf16: halves HBM traffic (the kernel
    is memory-bound) at ~1e-2 worst-case relative error, inside 2e-2.
"""

import os
import numpy as np
from ml_dtypes import bfloat16

# The bass kernel executes through the axon PJRT backend; make sure jax can
# see it even if the caller pinned JAX_PLATFORMS (e.g. to "cpu").
_jp = os.environ.get("JAX_PLATFORMS")
if _jp is not None and _jp != "" and "axon" not in _jp:
    os.environ["JAX_PLATFORMS"] = "axon," + _jp

M, D, E, H = 65536, 768, 20, 64
N_CORES = 8
MS = M // N_CORES          # rows per core = 8192
GROUP = 512                # rows per PE matmul group
N_GROUPS = MS // GROUP     # 16
KCH = D // 128             # 6 contraction chunks
SG = 4                     # groups per output DMA
N_SG = N_GROUPS // SG      # 4
# Flat per-partition element layout on ONE queue: a 512-element const block
# (w1 chunks + paired W2, 1KB-aligned) followed by the 16 groups as [k][512].
# Small uniform pieces keep job-completion semaphores posting in stream
# order (big or late-queued jobs complete out of order and stall the PE);
# 1-group pieces at the ends give an early PE start and a short tail.
CB = 512                                  # const block elements/partition
GELEMS = KCH * GROUP                      # 3072 per group
CFLAT = CB + N_GROUPS * GELEMS            # 49664
PB = (0, CB + GELEMS, CB + 2 * GELEMS, CB + 4 * GELEMS, CB + 6 * GELEMS,
      CB + 8 * GELEMS, CB + 10 * GELEMS, CB + 12 * GELEMS, CB + 14 * GELEMS,
      CB + 15 * GELEMS, CFLAT)

_CACHE = {}


def _build():
    """Build + compile the 8-core SPMD bass program once per process."""
    if "nc" in _CACHE:
        return _CACHE["nc"]

    import concourse.mybir as mybir
    import concourse.tile as tile
    from concourse import bacc

    F32 = mybir.dt.float32
    BF16 = mybir.dt.bfloat16

    nc = bacc.Bacc("TRN2", target_bir_lowering=False, debug=False,
                   enable_asserts=False, num_devices=N_CORES)

    xt_d = nc.dram_tensor("xt", [128, CFLAT], BF16,
                          kind="ExternalInput").ap()
    out_d = nc.dram_tensor("out", [MS], F32, kind="ExternalOutput").ap()
    # m = ((s*2 + p)*2 + h)*512 + c: pair rows h land interleaved in DRAM
    out_r = out_d.rearrange("(s p h c) -> s h p c", s=N_SG, p=2, h=2)

    with tile.TileContext(nc) as tc:
        with (
            tc.tile_pool(name="consts", bufs=1) as cpool,
            tc.tile_pool(name="xin", bufs=1) as px,
            tc.tile_pool(name="ht", bufs=6) as pht,
            tc.tile_pool(name="osb", bufs=2) as posb,
            tc.tile_pool(name="psz", bufs=5, space="PSUM") as psz,
            tc.tile_pool(name="pss", bufs=3, space="PSUM") as pss,
        ):
            def load_piece(i):
                lo, hi = PB[i], PB[i + 1]
                xk = px.tile([128, hi - lo], BF16, tag=f"xin{i}")
                nc.sync.dma_start(xk[:], xt_d[:, lo:hi])
                return xk

            tiles = [load_piece(i) for i in range(len(PB) - 1)]
            # consts live at the head of piece 0: w1 chunk k at [64k, 64k+64),
            # the paired W2 column pair at [384, 386)
            w1t = tiles[0]
            wsc = w1t[:, KCH * H:KCH * H + 2]

            def place(g):
                """(piece tile, local element offset) for group g."""
                if g == 0:
                    return tiles[0], CB
                if g == 1:
                    return tiles[1], 0
                if g <= 13:
                    return tiles[2 + (g - 2) // 2], ((g - 2) % 2) * GELEMS
                return tiles[8 + (g - 14)], 0

            osb_tiles = {}
            pending = []

            def emit_pair(gp, ht):
                # one matmul produces both groups' scores: [2, GROUP] PSUM
                sc = pss.tile([2, GROUP], F32, tag="pss")
                nc.tensor.matmul(sc[:], wsc, ht[:], start=True, stop=True)
                pr, sq = gp % 2, gp // 2
                if pr == 0:
                    osb_t = posb.tile([2, 2, GROUP], F32, tag="osb")
                    osb_tiles[sq] = osb_t
                nc.scalar.copy(osb_tiles[sq][:, pr, :], sc[:])
                if pr == 1:
                    osb_t = osb_tiles.pop(sq)
                    nc.gpsimd.dma_start(out_r[sq], osb_t[:])

            zpair = None
            for g in range(N_GROUPS):
                xk, loc = place(g)
                if g % 2 == 0:
                    zpair = psz.tile([128, GROUP], F32, tag="psz")
                zt = zpair[64 * (g % 2):64 * (g % 2) + H, :]
                for k in range(KCH):
                    nc.tensor.matmul(zt, w1t[:, H * k:H * (k + 1)],
                                     xk[:, loc + k * GROUP:
                                         loc + (k + 1) * GROUP],
                                     start=(k == 0), stop=(k == KCH - 1))
                if g % 2 == 1:
                    # one relu covers the pair; vector engine (no
                    # activation-table load), scalar keeps the copies
                    ht = pht.tile([128, GROUP], BF16, tag="ht")
                    nc.vector.tensor_scalar_max(ht[:], zpair[:], 0.0)
                    pending.append((g // 2, ht))
                    if len(pending) > 1:
                        emit_pair(*pending.pop(0))
            while pending:
                emit_pair(*pending.pop(0))

    nc.compile()
    _CACHE["nc"] = nc
    return nc


_BOUNDS = np.array([1, 2, 3, 4, 5, 8, 16, 32, 64], np.int64)


def _bucket(c):
    """Identity buckets for c<=4, log2 buckets above, clamped to [0, 9].
    Integer-exact equivalent of the reference's float bucketing."""
    return np.searchsorted(_BOUNDS, np.asarray(c, np.int64), side="right")


def _prepare_maps(ment_emb, mem_vectors, dist_table, counter_table,
                  W1, b1, W2, b2, ent_counter, last_mention_start, ment_start):
    f64 = np.float64
    ment = np.asarray(ment_emb, f64)
    W1 = np.asarray(W1, f64)

    W1m, W1r, W1h = W1[0:D], W1[D:2 * D], W1[2 * D:3 * D]
    W1d, W1c = W1[3 * D:3 * D + E], W1[3 * D + E:3 * D + 2 * E]

    W = W1m + ment[:, None] * W1h                       # [768, 64]
    bias = np.asarray(b1, f64) + ment @ W1r             # [64]
    Td = np.asarray(dist_table, f64) @ W1d + bias       # [10, 64]
    Tc = np.asarray(counter_table, f64) @ W1c           # [10, 64]
    # Wp = W (W^T W)^{-1}; W^T (x + Wp t) = W^T x + t exactly
    Wp = np.linalg.solve(W.T @ W, W.T).T                # [768, 64]
    T_all = (Td[:, None, :] + Tc[None, :, :]).reshape(100, H)
    Delta = (T_all @ Wp.T).astype(np.float32)           # [100, 768]

    cnt = np.asarray(ent_counter, np.int64)
    dist = int(np.asarray(ment_start)) - np.asarray(last_mention_start,
                                                    np.int64)
    idx = _bucket(dist) * 10 + _bucket(cnt)             # [M]

    mem = np.asarray(mem_vectors, np.float32)
    xp = mem + Delta[idx]                               # [M, 768] f32
    cb = np.zeros((128, CB), bfloat16)
    cb[:, :KCH * H] = (W.astype(np.float32).astype(bfloat16)
                       .reshape(KCH, 128, H).transpose(1, 0, 2)
                       .reshape(128, KCH * H))
    w2b = np.asarray(W2, np.float32).astype(bfloat16).reshape(H)
    cb[0:H, KCH * H] = w2b
    cb[H:2 * H, KCH * H + 1] = w2b

    in_maps = []
    for c in range(N_CORES):
        sl = slice(c * MS, (c + 1) * MS)
        a = (xp[sl].T.reshape(KCH, 128, N_GROUPS, GROUP)
             .transpose(1, 2, 0, 3).reshape(128, -1).astype(bfloat16))
        xt = np.ascontiguousarray(np.concatenate([cb, a], 1))
        in_maps.append(dict(xt=xt))

    _CACHE["mask"] = cnt == 0
    _CACHE["b2"] = float(np.asarray(b2, np.float64).reshape(-1)[0])
    return in_maps


def _postprocess(results):
    out = np.empty(M + 1, np.float32)
    for c in range(N_CORES):
        out[c * MS:(c + 1) * MS] = results[c]["out"]
    out[:M] += _CACHE["b2"]
    out[:M][_CACHE["mask"]] = -10000.0
    out[M] = 0.0
    return out


def run_spmd(in_maps, trace=False):
    from concourse.bass_utils import run_bass_kernel_spmd
    nc = _build()
    return run_bass_kernel_spmd(nc, in_maps, list(range(N_CORES)), trace=trace)


def kernel(**inputs):
    in_maps = _prepare_maps(**inputs)
    res = run_spmd(in_maps, trace=False)
    return _postprocess(res.results)
